# revision 8
# baseline (speedup 1.0000x reference)
"""PoH block (3-iter transformer block) on 8 trn2 NeuronCores.

Sharding: pure data-parallel over batch (B=8 -> 1 element/core), weights
replicated, zero collectives. Per-core ~73 GFLOP, compute-bound.

Mixed precision tuned so max-rel error stays ~2e-3 (gate 2e-2):
- q/k/v projections: fp8e4m3 DoubleRow with hi/lo-split weights (weight
  quantization error cancelled; only z-fp8 error remains) -> 2x PE.
- scores / PV: fp8 DoubleRow with a zero second k-tile (K=64 / K=128
  contractions at 0.5 cyc/row) -> 2x PE. exp outputs fp8 directly with
  the P-scale folded into the exp bias; the softmax denominator rides as
  a scaled ones-column of V so all fp8 scales cancel exactly in P*V/den.
- out-proj and FFN: bf16 weights/activations (full PE rate, half DMA);
  these are the error-critical matmuls so they stay high precision.
Softmax needs no max-subtraction (|logits| <= ~3.2 by construction;
exp*4 stays inside fp8e4m3 range).

The attention inner loop is exp-throughput-bound on the Activation
engine, so the PE work of the NEXT head group's q/k/v projections is
interleaved into the exp-wait slots (the PE executes in order; program
order is the schedule). PV lags exp by one s-chunk and the softmax
normalization of each pass is deferred into the next pass.
"""

import numpy as np
import ml_dtypes
from collections import deque
from contextlib import ExitStack

import concourse.bacc as bacc
import concourse.mybir as mybir
import concourse.tile as tile
from concourse.bass_utils import run_bass_kernel_spmd
from concourse.masks import make_identity

F32 = mybir.dt.float32
F32R = mybir.dt.float32r
BF16 = mybir.dt.bfloat16
FP8 = mybir.dt.float8e4
AF = mybir.ActivationFunctionType
OP = mybir.AluOpType
DRM = mybir.MatmulPerfMode.DoubleRow
NPF8 = ml_dtypes.float8_e4m3

D = 1024
H = 16
DH = 64
DF = 4096
B = 8
ITERS = 3
EPS = 1e-5

SW = 256.0   # weight scale (hi/lo fp8)
SZ = 16.0    # z scale into zt8
SQ = 16.0    # q/k scale into qt8/kt8
SPE = 4.0    # exp output scale (bias ln(SPE)); max logit ~3.2 -> exp*4 < 120
SPV = 16.0   # v scale into vg8 (ones column = SPV, cancels in P*V/den)
QK_DESCALE = SQ / (SZ * SW)        # psum(q*SZ*SW) -> q*SQ
V_DESCALE = SPV / (SZ * SW)        # psum(v*SZ*SW) -> v*SPV
EXP_SCALE = 0.125 / (SQ * SQ)      # psum(q*SQ . k*SQ) -> logits

_CACHE = {}


def build(T=1024):
    nc = bacc.Bacc("TRN2", target_bir_lowering=False, dynamic_dma_scratch_size=4096)

    NT1 = T // 128   # t chunks of 128
    NT5 = T // 512   # t chunks of 512
    TQB = NT5        # 512-col blocks per score/exp tile
    ND = D // 128    # 8
    NF = DF // 128   # 32

    z_in = nc.dram_tensor("z_in", [T, D], F32, kind="ExternalInput")
    wq8 = nc.dram_tensor("wq8", [D, 2, D], FP8, kind="ExternalInput")
    wk8 = nc.dram_tensor("wk8", [D, 2, D], FP8, kind="ExternalInput")
    wv8 = nc.dram_tensor("wv8", [D, 2, D], FP8, kind="ExternalInput")
    wo = nc.dram_tensor("wo", [D, D], BF16, kind="ExternalInput")
    w1 = nc.dram_tensor("w1", [D, DF], BF16, kind="ExternalInput")
    w2 = nc.dram_tensor("w2", [DF, D], BF16, kind="ExternalInput")
    z_out = nc.dram_tensor("z_out", [T, D], F32, kind="ExternalOutput")
    z_ln1 = [nc.dram_tensor(f"z_ln1_{i}", [T, D], F32) for i in range(2)]
    z_ln2 = [nc.dram_tensor(f"z_ln2_{i}", [T, D], F32) for i in range(2)]

    with ExitStack() as ctx:
        tc = ctx.enter_context(tile.TileContext(nc))
        ctx.enter_context(nc.allow_low_precision(reason="fp8/bf16 pipeline"))
        singles = ctx.enter_context(tc.tile_pool(name="singles", bufs=1))
        work = ctx.enter_context(tc.tile_pool(name="work", bufs=2))
        stats = ctx.enter_context(tc.tile_pool(name="stats", bufs=3))
        wg_p = ctx.enter_context(tc.tile_pool(name="wg", bufs=6))
        psum = ctx.enter_context(tc.tile_pool(name="psum", bufs=2, space="PSUM"))

        ident = singles.tile([128, 128], F32, name="ident")
        make_identity(nc, ident)
        ones_row_f = singles.tile([1, 64], F32, name="ones_row_f")
        nc.vector.memset(ones_row_f, 1.0)
        ones_row = singles.tile([1, 64], F32R, name="ones_row")
        nc.vector.tensor_copy(out=ones_row, in_=ones_row_f)
        eps_t = singles.tile([128, 1], F32, name="eps_t")
        nc.vector.memset(eps_t, EPS)
        lnspe = singles.tile([128, 1], F32, name="lnspe")
        nc.vector.memset(lnspe, float(np.log(SPE)))

        # persistent data tiles
        zt8 = singles.tile([128, ND, T], FP8, name="zt8")
        z1t = singles.tile([128, ND, T], BF16, name="z1t")
        outcat = singles.tile([128, ND, T], BF16, name="outcat")
        wo_sb = singles.tile([128, ND, D], BF16, name="wo_sb")

        # double-buffered per-group q/k/v (second k-tile slot zeroed once)
        qts, kts, vgs = [], [], []
        for i in range(2):
            qt = singles.tile([128, 2, T], FP8, name=f"qt{i}")
            kt = singles.tile([128, 2, 2, T], FP8, name=f"kt{i}")
            vg = singles.tile([128, 2, NT1, 4, 65], FP8, name=f"vg{i}")
            nc.vector.memset(kt[:, 1, :, :], 0.0)
            nc.vector.memset(vg[:, 1, :, :, :], 0.0)
            nc.vector.memset(vg[:, 0, :, :, 64:65], SPV)
            qts.append(qt); kts.append(kt); vgs.append(vg)

        # ring of exp tiles (second k-tile slot is never-written junk;
        # zeroed once so it can't hold NaN bit patterns)
        ET_RING = 4
        ets = []
        for i in range(ET_RING):
            et = singles.tile([128, 2, TQB, 512], FP8, name=f"et{i}")
            nc.vector.memset(et[:, 1, :, :], 0.0)
            ets.append(et)
        et_idx = [0]

        def next_et():
            et = ets[et_idx[0] % ET_RING]
            et_idx[0] += 1
            return et

        def layernorm_tile(ln_in, z_new):
            """ln_in [128, D] f32 -> z_new [128, D] f32 (gamma=1, beta=0)."""
            st = stats.tile([128, 2, 6], F32, name="bn", tag="bn")
            for c in range(2):
                nc.vector.bn_stats(out=st[:, c, :], in_=ln_in[:, c * 512:(c + 1) * 512])
            mv = stats.tile([128, 2], F32, name="mv", tag="mv")
            nc.vector.bn_aggr(out=mv, in_=st)
            rstd = stats.tile([128, 1], F32, name="rstd", tag="rstd")
            nc.scalar.activation(out=rstd, in_=mv[:, 1:2], func=AF.Sqrt, bias=eps_t, scale=1.0)
            nc.vector.reciprocal(out=rstd, in_=rstd)
            nc.vector.tensor_scalar(out=z_new, in0=ln_in, scalar1=mv[:, 0:1], scalar2=rstd,
                                    op0=OP.subtract, op1=OP.mult)

        def transpose_zrow(src_tile, tp, dst, dtype_scale=None):
            """src_tile [128, D] f32 (t-chunk tp) -> dst[:, dp, tp*128:+128]."""
            for dp in range(ND):
                pt = psum.tile([128, 128], F32, name="pt", tag="acc")
                nc.tensor.transpose(pt, in_=src_tile[:, dp * 128:(dp + 1) * 128], identity=ident)
                sl = dst[:, dp, tp * 128:(tp + 1) * 128]
                if dtype_scale is None:
                    nc.vector.tensor_copy(out=sl, in_=pt)
                else:
                    nc.vector.tensor_scalar_mul(out=sl, in0=pt, scalar1=dtype_scale)

        # ---- q/k/v projection machinery ----
        wgt_tiles = {}

        def emit_wgt_dma(g):
            cs = g * 256
            for pname, wdram in (("q", wq8), ("k", wk8), ("v", wv8)):
                wgt = wg_p.tile([128, ND, 2, 256], FP8, name="wgt", tag="wgt")
                for dp in range(ND):
                    nc.sync.dma_start(out=wgt[:, dp, :, :],
                                      in_=wdram[dp * 128:(dp + 1) * 128, :, cs:cs + 256])
                wgt_tiles[(g, pname)] = wgt

        def qkv_units(g):
            """PE filler units: q/k per (pname, hp, tq); v per sp."""
            units = []
            qt, kt, vg = qts[g % 2], kts[g % 2], vgs[g % 2]
            for pname in ("q", "k"):
                dst = qt if pname == "q" else kt
                for hp in range(2):
                    for tq in range(NT5):
                        def u(pname=pname, hp=hp, tq=tq, dst=dst, g=g):
                            wgt = wgt_tiles[(g, pname)]
                            acc = psum.tile([128, 512], F32, name="acq", tag="acc")
                            for dp in range(ND):
                                zb = zt8[:, dp, tq * 512:(tq + 1) * 512]
                                zb = zb[:, None, :].broadcast_to([128, 2, 512])
                                nc.tensor.matmul(acc,
                                                 lhsT=wgt[:, dp, :, hp * 128:(hp + 1) * 128],
                                                 rhs=zb, perf_mode=DRM,
                                                 start=(dp == 0), stop=(dp == ND - 1))
                            if pname == "q":
                                sl = dst[:, hp, tq * 512:(tq + 1) * 512]
                            else:
                                sl = dst[:, 0, hp, tq * 512:(tq + 1) * 512]
                            nc.vector.tensor_scalar_mul(out=sl, in0=acc, scalar1=QK_DESCALE)
                        units.append(u)
            for sp in range(NT1):
                def u(sp=sp, vg=vg, g=g):
                    wgt = wgt_tiles[(g, "v")]
                    acc = psum.tile([128, 256], F32, name="acv", tag="acc")
                    for dp in range(ND):
                        zb = zt8[:, dp, sp * 128:(sp + 1) * 128]
                        zb = zb[:, None, :].broadcast_to([128, 2, 128])
                        nc.tensor.matmul(acc, lhsT=zb, rhs=wgt[:, dp, :, :],
                                         perf_mode=DRM,
                                         start=(dp == 0), stop=(dp == ND - 1))
                    nc.vector.tensor_scalar_mul(
                        out=vg[:, 0, sp, :, 0:64],
                        in0=acc.rearrange("p (h e) -> p h e", e=64),
                        scalar1=V_DESCALE)
                units.append(u)
            return units

        pending_norm = []

        def attention_pass(g, hp, hh, fillers):
            qt, kt, vg = qts[g % 2], kts[g % 2], vgs[g % 2]
            r0 = hh * 64
            hep = g * 2 + hp
            pv = psum.tile([65, TQB, 512], F32, name="pv", tag="pv", bufs=1)
            ets_local = []

            def pv_mm(sp):
                for tq in range(TQB):
                    nc.tensor.matmul(pv[:, tq, :],
                                     lhsT=vg[:, :, sp, hp * 2 + hh, :],
                                     rhs=ets_local[sp][:, :, tq, :],
                                     perf_mode=DRM,
                                     start=(sp == 0), stop=(sp == NT1 - 1))

            for sp in range(NT1):
                sc = psum.tile([128, TQB, 512], F32, name="sc", tag="sc")
                for tq in range(TQB):
                    nc.tensor.matmul(
                        sc[:, tq, :],
                        lhsT=kt[r0:r0 + 64, :, hp, sp * 128:(sp + 1) * 128],
                        rhs=qt[r0:r0 + 64, hp, tq * 512:(tq + 1) * 512][:, None, :]
                            .broadcast_to([64, 2, 512]),
                        perf_mode=DRM, start=True, stop=True)
                et = next_et()
                nc.scalar.activation(out=et[:, 0, :, :], in_=sc, func=AF.Exp,
                                     bias=lnspe, scale=EXP_SCALE)
                ets_local.append(et)
                if sp == 1 and pending_norm:
                    pending_norm.pop()()
                if sp % 2 == 1 and fillers:
                    fillers.popleft()()
                if sp >= 1:
                    pv_mm(sp - 1)
            pv_mm(NT1 - 1)
            rec = stats.tile([1, TQB, 512], F32R, name="rec", tag="rec")
            nc.vector.reciprocal(out=rec, in_=pv[64:65, :, :])

            def norm(pv=pv, rec=rec, r0=r0, hep=hep):
                for tq in range(TQB):
                    pb = psum.tile([64, 512], F32, name="pb", tag="acc")
                    nc.tensor.matmul(pb, lhsT=ones_row, rhs=rec[:, tq, :],
                                     start=True, stop=True)
                    rb = stats.tile([64, 512], F32, name="rb", tag="rb", bufs=2)
                    nc.vector.tensor_copy(out=rb, in_=pb)
                    nc.vector.tensor_mul(
                        out=outcat[r0:r0 + 64, hep, tq * 512:(tq + 1) * 512],
                        in0=pv[0:64, tq, :], in1=rb)
            pending_norm.append(norm)

        # ---- initial zt8 + first group's weights ----
        emit_wgt_dma(0)
        for tp in range(NT1):
            zi = work.tile([128, D], F32, name="zi", tag="zres", bufs=3)
            nc.sync.dma_start(out=zi, in_=z_in[tp * 128:(tp + 1) * 128, :])
            transpose_zrow(zi, tp, zt8, dtype_scale=SZ)

        for it in range(ITERS):
            # ======== attention ========
            for u in qkv_units(0):
                u()
            for g in range(4):
                if g < 3:
                    emit_wgt_dma(g + 1)
                    fillers = deque(qkv_units(g + 1))
                else:
                    fillers = deque()
                for hp in range(2):
                    for hh in range(2):
                        attention_pass(g, hp, hh, fillers)
                while fillers:
                    fillers.popleft()()
            while pending_norm:
                pending_norm.pop()()

            # ======== out-proj + residual + LN1 ========
            if it == 0:
                for dp in range(ND):
                    nc.sync.dma_start(out=wo_sb[:, dp, :],
                                      in_=wo[dp * 128:(dp + 1) * 128, :])
            z_prev = z_in if it == 0 else z_ln2[it - 1]
            dst = z_out if it == ITERS - 1 else z_ln1[it]
            for tp in range(NT1):
                acc = psum.tile([128, 2, 512], F32, name="aao", tag="sc")
                for hep in range(ND):
                    for dq in range(2):
                        nc.tensor.matmul(acc[:, dq, :],
                                         lhsT=outcat[:, hep, tp * 128:(tp + 1) * 128],
                                         rhs=wo_sb[:, hep, dq * 512:(dq + 1) * 512],
                                         start=(hep == 0), stop=(hep == ND - 1))
                zp = work.tile([128, D], F32, name="zp", tag="zres", bufs=3)
                nc.sync.dma_start(out=zp, in_=z_prev[tp * 128:(tp + 1) * 128, :])
                ln_in = work.tile([128, D], F32, name="ln_in", tag="ln_in", bufs=3)
                for dq in range(2):
                    nc.vector.tensor_add(out=ln_in[:, dq * 512:(dq + 1) * 512],
                                         in0=zp[:, dq * 512:(dq + 1) * 512],
                                         in1=acc[:, dq, :])
                z_new = work.tile([128, D], F32, name="z_new", tag="z_new", bufs=3)
                layernorm_tile(ln_in, z_new)
                nc.sync.dma_start(out=dst[tp * 128:(tp + 1) * 128, :], in_=z_new)
                if it < ITERS - 1:
                    transpose_zrow(z_new, tp, z1t)

            if it == ITERS - 1:
                break

            # ======== FFN ========
            with tc.tile_pool(name="htp", bufs=1) as ht_p, \
                 tc.tile_pool(name="w1p", bufs=2) as w1_p, \
                 tc.tile_pool(name="w2p", bufs=3) as w2_p:
                emit_wgt_dma(0)  # prefetch next iteration's first group
                for th in range(NT5):
                    ts0 = th * 512
                    ht = ht_p.tile([128, NF, 512], BF16, name="ht", tag="ht")
                    for fblk in range(8):
                        w1c = w1_p.tile([128, ND, 512], BF16, name="w1c", tag="w1c")
                        for dp in range(ND):
                            nc.sync.dma_start(out=w1c[:, dp, :],
                                              in_=w1[dp * 128:(dp + 1) * 128,
                                                     fblk * 512:(fblk + 1) * 512])
                        for fi in range(4):
                            fc = fblk * 4 + fi
                            acc = psum.tile([128, 512], F32, name="ah", tag="acc")
                            for dp in range(ND):
                                nc.tensor.matmul(acc,
                                                 lhsT=w1c[:, dp, fi * 128:(fi + 1) * 128],
                                                 rhs=z1t[:, dp, ts0:ts0 + 512],
                                                 start=(dp == 0), stop=(dp == ND - 1))
                            nc.scalar.activation(out=ht[:, fc, :], in_=acc, func=AF.Relu)
                    # h2: stream w2 once per th; 4 t-chunk accumulators spread
                    # across the psum tags (sc, sc, pv, acc+acc) = 8 banks.
                    acc_sc0 = psum.tile([128, 2, 512], F32, name="af0", tag="sc")
                    acc_sc1 = psum.tile([128, 2, 512], F32, name="af1", tag="sc")
                    acc_pv = psum.tile([128, 2, 512], F32, name="af2", tag="pv", bufs=1)
                    acc_a0 = psum.tile([128, 512], F32, name="af3a", tag="acc")
                    acc_a1 = psum.tile([128, 512], F32, name="af3b", tag="acc")
                    acc_of = {0: (lambda dq: acc_sc0[:, dq, :]),
                              1: (lambda dq: acc_sc1[:, dq, :]),
                              2: (lambda dq: acc_pv[:, dq, :]),
                              3: (lambda dq: (acc_a0, acc_a1)[dq][:, :])}
                    for fc in range(NF):
                        w2c = w2_p.tile([128, D], BF16, name="w2c", tag="w2c")
                        nc.sync.dma_start(out=w2c, in_=w2[fc * 128:(fc + 1) * 128, :])
                        for ti in range(4):
                            for dq in range(2):
                                nc.tensor.matmul(acc_of[ti](dq),
                                                 lhsT=ht[:, fc, ti * 128:(ti + 1) * 128],
                                                 rhs=w2c[:, dq * 512:(dq + 1) * 512],
                                                 start=(fc == 0), stop=(fc == NF - 1))
                    for ti in range(4):
                        tp = th * 4 + ti
                        zp = work.tile([128, D], F32, name="zp2", tag="zres", bufs=3)
                        nc.sync.dma_start(out=zp, in_=z_ln1[it][tp * 128:(tp + 1) * 128, :])
                        ln_in = work.tile([128, D], F32, name="ln_in2", tag="ln_in", bufs=3)
                        for dq in range(2):
                            nc.vector.tensor_add(out=ln_in[:, dq * 512:(dq + 1) * 512],
                                                 in0=zp[:, dq * 512:(dq + 1) * 512],
                                                 in1=acc_of[ti](dq))
                        z_new = work.tile([128, D], F32, name="z_new2", tag="z_new", bufs=3)
                        layernorm_tile(ln_in, z_new)
                        nc.sync.dma_start(out=z_ln2[it][tp * 128:(tp + 1) * 128, :], in_=z_new)
                        transpose_zrow(z_new, tp, zt8, dtype_scale=SZ)

    nc.compile()
    return nc


def _prep_w_hilo(w):
    """[D, D] f32 -> [D, 2, D] fp8 hi/lo at scale SW."""
    ws = np.asarray(w, dtype=np.float32) * SW
    hi = ws.astype(NPF8)
    lo = (ws - hi.astype(np.float32)).astype(NPF8)
    return np.ascontiguousarray(np.stack([hi, lo], axis=1))


def kernel(**inputs):
    z = np.asarray(inputs["z"], dtype=np.float32)
    for nm in ("bq", "bk", "bv", "bo", "b1", "b2", "be1", "be2"):
        assert not np.any(np.asarray(inputs[nm])), f"{nm} must be zero (specialized kernel)"
    for nm in ("g1", "g2"):
        assert np.all(np.asarray(inputs[nm]) == 1.0), f"{nm} must be ones (specialized kernel)"

    def flat(w):
        return np.asarray(w).transpose(1, 0, 2).reshape(D, D).astype(np.float32)

    wq_ = _prep_w_hilo(flat(inputs["Wq"]))
    wk_ = _prep_w_hilo(flat(inputs["Wk"]))
    wv_ = _prep_w_hilo(flat(inputs["Wv"]))
    wo_ = np.ascontiguousarray(np.asarray(inputs["Wo"], dtype=np.float32).astype(ml_dtypes.bfloat16))
    w1_ = np.ascontiguousarray(np.asarray(inputs["W1"], dtype=np.float32).astype(ml_dtypes.bfloat16))
    w2_ = np.ascontiguousarray(np.asarray(inputs["W2"], dtype=np.float32).astype(ml_dtypes.bfloat16))

    T = z.shape[1]
    if T not in _CACHE:
        _CACHE[T] = build(T)
    nc = _CACHE[T]

    in_maps = [{"z_in": np.ascontiguousarray(z[c]), "wq8": wq_, "wk8": wk_, "wv8": wv_,
                "wo": wo_, "w1": w1_, "w2": w2_} for c in range(B)]
    res = run_bass_kernel_spmd(nc, in_maps, core_ids=list(range(B)))
    return np.stack([res.results[c]["z_out"] for c in range(B)]).astype(np.float32)


# revision 10
# speedup vs baseline: 1.0012x; 1.0012x over previous
"""PoH block (3-iter transformer block) on 8 trn2 NeuronCores.

Sharding: pure data-parallel over batch (B=8 -> 1 element/core), weights
replicated, zero collectives. Per-core ~73 GFLOP, compute-bound.

Mixed precision tuned so max-rel error stays ~2e-3 (gate 2e-2):
- q/k/v projections: fp8e4m3 DoubleRow with hi/lo-split weights (weight
  quantization error cancelled; only z-fp8 error remains) -> 2x PE.
- scores / PV: fp8 DoubleRow with a zero second k-tile (K=64 / K=128
  contractions at 0.5 cyc/row) -> 2x PE. exp outputs fp8 directly with
  the P-scale folded into the exp bias; the softmax denominator rides as
  a scaled ones-column of V so all fp8 scales cancel exactly in P*V/den.
- out-proj and FFN: bf16 weights/activations (full PE rate, half DMA);
  these are the error-critical matmuls so they stay high precision.
Softmax needs no max-subtraction (|logits| <= ~3.2 by construction;
exp*4 stays inside fp8e4m3 range).

The attention inner loop is exp-throughput-bound on the Activation
engine, so the PE work of the NEXT head group's q/k/v projections is
interleaved into the exp-wait slots (the PE executes in order; program
order is the schedule). PV lags exp by one s-chunk and the softmax
normalization of each pass is deferred into the next pass.
"""

import numpy as np
import ml_dtypes
from collections import deque
from contextlib import ExitStack

import concourse.bacc as bacc
import concourse.mybir as mybir
import concourse.tile as tile
from concourse.bass_utils import run_bass_kernel_spmd
from concourse.masks import make_identity

F32 = mybir.dt.float32
F32R = mybir.dt.float32r
BF16 = mybir.dt.bfloat16
FP8 = mybir.dt.float8e4
AF = mybir.ActivationFunctionType
OP = mybir.AluOpType
DRM = mybir.MatmulPerfMode.DoubleRow
NPF8 = ml_dtypes.float8_e4m3

D = 1024
H = 16
DH = 64
DF = 4096
B = 8
ITERS = 3
EPS = 1e-5

SW = 256.0   # weight scale (hi/lo fp8)
SZ = 16.0    # z scale into zt8
SQ = 16.0    # q/k scale into qt8/kt8
SPE = 4.0    # exp output scale (bias ln(SPE)); max logit ~3.2 -> exp*4 < 120
SPV = 16.0   # v scale into vg8 (ones column = SPV, cancels in P*V/den)
QK_DESCALE = SQ / (SZ * SW)        # psum(q*SZ*SW) -> q*SQ
V_DESCALE = SPV / (SZ * SW)        # psum(v*SZ*SW) -> v*SPV
EXP_SCALE = 0.125 / (SQ * SQ)      # psum(q*SQ . k*SQ) -> logits

_CACHE = {}


def build(T=1024):
    nc = bacc.Bacc("TRN2", target_bir_lowering=False, dynamic_dma_scratch_size=4096)

    NT1 = T // 128   # t chunks of 128
    NT5 = T // 512   # t chunks of 512
    TQB = NT5        # 512-col blocks per score/exp tile
    ND = D // 128    # 8
    NF = DF // 128   # 32

    z_in = nc.dram_tensor("z_in", [T, D], F32, kind="ExternalInput")
    wq8 = nc.dram_tensor("wq8", [D, 2, D], FP8, kind="ExternalInput")
    wk8 = nc.dram_tensor("wk8", [D, 2, D], FP8, kind="ExternalInput")
    wv8 = nc.dram_tensor("wv8", [D, 2, D], FP8, kind="ExternalInput")
    wo = nc.dram_tensor("wo", [D, D], BF16, kind="ExternalInput")
    w1 = nc.dram_tensor("w1", [D, DF], BF16, kind="ExternalInput")
    w2 = nc.dram_tensor("w2", [DF, D], BF16, kind="ExternalInput")
    z_out = nc.dram_tensor("z_out", [T, D], F32, kind="ExternalOutput")
    z_ln1 = [nc.dram_tensor(f"z_ln1_{i}", [T, D], F32) for i in range(2)]
    z_ln2 = [nc.dram_tensor(f"z_ln2_{i}", [T, D], F32) for i in range(2)]

    with ExitStack() as ctx:
        tc = ctx.enter_context(tile.TileContext(nc))
        ctx.enter_context(nc.allow_low_precision(reason="fp8/bf16 pipeline"))
        singles = ctx.enter_context(tc.tile_pool(name="singles", bufs=1))
        work = ctx.enter_context(tc.tile_pool(name="work", bufs=2))
        stats = ctx.enter_context(tc.tile_pool(name="stats", bufs=3))
        wg_p = ctx.enter_context(tc.tile_pool(name="wg", bufs=6))
        psum = ctx.enter_context(tc.tile_pool(name="psum", bufs=2, space="PSUM"))

        ident = singles.tile([128, 128], F32, name="ident")
        make_identity(nc, ident)
        ones_row_f = singles.tile([1, 64], F32, name="ones_row_f")
        nc.vector.memset(ones_row_f, 1.0)
        ones_row = singles.tile([1, 64], F32R, name="ones_row")
        nc.vector.tensor_copy(out=ones_row, in_=ones_row_f)
        eps_t = singles.tile([128, 1], F32, name="eps_t")
        nc.vector.memset(eps_t, EPS)
        lnspe = singles.tile([128, 1], F32, name="lnspe")
        nc.vector.memset(lnspe, float(np.log(SPE)))

        # persistent data tiles
        zt8 = singles.tile([128, ND, T], FP8, name="zt8")
        z1t = singles.tile([128, ND, T], BF16, name="z1t")
        outcat = singles.tile([128, ND, T], BF16, name="outcat")
        wo_sb = singles.tile([128, ND, D], BF16, name="wo_sb")

        # double-buffered per-group q/k/v (second k-tile slot zeroed once)
        qts, kts, vgs = [], [], []
        for i in range(2):
            qt = singles.tile([128, 2, T], FP8, name=f"qt{i}")
            kt = singles.tile([128, 2, 2, T], FP8, name=f"kt{i}")
            vg = singles.tile([128, 2, NT1, 4, 65], FP8, name=f"vg{i}")
            nc.vector.memset(kt[:, 1, :, :], 0.0)
            nc.vector.memset(vg[:, 1, :, :, :], 0.0)
            nc.vector.memset(vg[:, 0, :, :, 64:65], SPV)
            qts.append(qt); kts.append(kt); vgs.append(vg)

        # ring of exp tiles (second k-tile slot is never-written junk;
        # zeroed once so it can't hold NaN bit patterns)
        ET_RING = 4
        ets = []
        for i in range(ET_RING):
            et = singles.tile([128, 2, TQB, 512], FP8, name=f"et{i}")
            nc.vector.memset(et[:, 1, :, :], 0.0)
            ets.append(et)
        et_idx = [0]

        def next_et():
            et = ets[et_idx[0] % ET_RING]
            et_idx[0] += 1
            return et

        def layernorm_tile(ln_in, z_new):
            """ln_in [128, D] f32 -> z_new [128, D] f32 (gamma=1, beta=0)."""
            st = stats.tile([128, 2, 6], F32, name="bn", tag="bn")
            for c in range(2):
                nc.vector.bn_stats(out=st[:, c, :], in_=ln_in[:, c * 512:(c + 1) * 512])
            mv = stats.tile([128, 2], F32, name="mv", tag="mv")
            nc.vector.bn_aggr(out=mv, in_=st)
            rstd = stats.tile([128, 1], F32, name="rstd", tag="rstd")
            nc.scalar.activation(out=rstd, in_=mv[:, 1:2], func=AF.Sqrt, bias=eps_t, scale=1.0)
            nc.vector.reciprocal(out=rstd, in_=rstd)
            nc.vector.tensor_scalar(out=z_new, in0=ln_in, scalar1=mv[:, 0:1], scalar2=rstd,
                                    op0=OP.subtract, op1=OP.mult)

        def transpose_zrow(src_tile, tp, dst, dtype_scale=None):
            """src_tile [128, D] f32 (t-chunk tp) -> dst[:, dp, tp*128:+128]."""
            for dp in range(ND):
                pt = psum.tile([128, 128], F32, name="pt", tag="acc")
                nc.tensor.transpose(pt, in_=src_tile[:, dp * 128:(dp + 1) * 128], identity=ident)
                sl = dst[:, dp, tp * 128:(tp + 1) * 128]
                if dtype_scale is None:
                    nc.vector.tensor_copy(out=sl, in_=pt)
                else:
                    nc.vector.tensor_scalar_mul(out=sl, in0=pt, scalar1=dtype_scale)

        # ---- q/k/v projection machinery ----
        wgt_tiles = {}

        def emit_wgt_dma(g):
            cs = g * 256
            for pname, wdram in (("q", wq8), ("k", wk8), ("v", wv8)):
                wgt = wg_p.tile([128, ND, 2, 256], FP8, name="wgt", tag="wgt")
                for dp in range(ND):
                    nc.sync.dma_start(out=wgt[:, dp, :, :],
                                      in_=wdram[dp * 128:(dp + 1) * 128, :, cs:cs + 256])
                wgt_tiles[(g, pname)] = wgt

        def qkv_units(g):
            """PE filler units: (th_ready, closure); q/k per (pname, hp, tq); v per sp."""
            units = []
            qt, kt, vg = qts[g % 2], kts[g % 2], vgs[g % 2]
            for pname in ("q", "k"):
                dst = qt if pname == "q" else kt
                for hp in range(2):
                    for tq in range(NT5):
                        def u(pname=pname, hp=hp, tq=tq, dst=dst, g=g):
                            wgt = wgt_tiles[(g, pname)]
                            acc = psum.tile([128, 512], F32, name="acq", tag="acc")
                            for dp in range(ND):
                                zb = zt8[:, dp, tq * 512:(tq + 1) * 512]
                                zb = zb[:, None, :].broadcast_to([128, 2, 512])
                                nc.tensor.matmul(acc,
                                                 lhsT=wgt[:, dp, :, hp * 128:(hp + 1) * 128],
                                                 rhs=zb, perf_mode=DRM,
                                                 start=(dp == 0), stop=(dp == ND - 1))
                            if pname == "q":
                                sl = dst[:, hp, tq * 512:(tq + 1) * 512]
                            else:
                                sl = dst[:, 0, hp, tq * 512:(tq + 1) * 512]
                            nc.vector.tensor_scalar_mul(out=sl, in0=acc, scalar1=QK_DESCALE)
                        units.append((tq, u))
            for sp in range(NT1):
                def u(sp=sp, vg=vg, g=g):
                    wgt = wgt_tiles[(g, "v")]
                    acc = psum.tile([128, 256], F32, name="acv", tag="acc")
                    for dp in range(ND):
                        zb = zt8[:, dp, sp * 128:(sp + 1) * 128]
                        zb = zb[:, None, :].broadcast_to([128, 2, 128])
                        nc.tensor.matmul(acc, lhsT=zb, rhs=wgt[:, dp, :, :],
                                         perf_mode=DRM,
                                         start=(dp == 0), stop=(dp == ND - 1))
                    nc.vector.tensor_scalar_mul(
                        out=vg[:, 0, sp, :, 0:64],
                        in0=acc.rearrange("p (h e) -> p h e", e=64),
                        scalar1=V_DESCALE)
                units.append((sp * 128 // 512, u))
            return units

        pending_norm = []
        pending_tail = []

        def attention_pass(g, hp, hh, fillers):
            qt, kt, vg = qts[g % 2], kts[g % 2], vgs[g % 2]
            r0 = hh * 64
            hep = g * 2 + hp
            pv = psum.tile([65, TQB, 512], F32, name="pv", tag="pv", bufs=1)
            ets_local = []

            def pv_mm(sp):
                for tq in range(TQB):
                    nc.tensor.matmul(pv[:, tq, :],
                                     lhsT=vg[:, :, sp, hp * 2 + hh, :],
                                     rhs=ets_local[sp][:, :, tq, :],
                                     perf_mode=DRM,
                                     start=(sp == 0), stop=(sp == NT1 - 1))

            for sp in range(NT1):
                sc = psum.tile([128, TQB, 512], F32, name="sc", tag="sc")
                for tq in range(TQB):
                    nc.tensor.matmul(
                        sc[:, tq, :],
                        lhsT=kt[r0:r0 + 64, :, hp, sp * 128:(sp + 1) * 128],
                        rhs=qt[r0:r0 + 64, hp, tq * 512:(tq + 1) * 512][:, None, :]
                            .broadcast_to([64, 2, 512]),
                        perf_mode=DRM, start=True, stop=True)
                et = next_et()
                nc.scalar.activation(out=et[:, 0, :, :], in_=sc, func=AF.Exp,
                                     bias=lnspe, scale=EXP_SCALE)
                ets_local.append(et)
                if sp == 0 and pending_tail:
                    pending_tail.pop()()
                if sp == 1 and pending_norm:
                    pending_norm.pop()()
                if sp % 2 == 1 and fillers:
                    fillers.popleft()[1]()
                if sp >= 2:
                    pv_mm(sp - 2)
            rec = stats.tile([1, TQB, 512], F32R, name="rec", tag="rec")

            def tail(pv=pv, rec=rec):
                pv_mm(NT1 - 2)
                pv_mm(NT1 - 1)
                nc.vector.reciprocal(out=rec, in_=pv[64:65, :, :])
            pending_tail.append(tail)

            def norm(pv=pv, rec=rec, r0=r0, hep=hep):
                for tq in range(TQB):
                    pb = psum.tile([64, 512], F32, name="pb", tag="acc")
                    nc.tensor.matmul(pb, lhsT=ones_row, rhs=rec[:, tq, :],
                                     start=True, stop=True)
                    rb = stats.tile([64, 512], F32, name="rb", tag="rb", bufs=2)
                    nc.vector.tensor_copy(out=rb, in_=pb)
                    nc.vector.tensor_mul(
                        out=outcat[r0:r0 + 64, hep, tq * 512:(tq + 1) * 512],
                        in0=pv[0:64, tq, :], in1=rb)
            pending_norm.append(norm)

        # ---- initial zt8 + first group's weights ----
        emit_wgt_dma(0)
        for tp in range(NT1):
            zi = work.tile([128, D], F32, name="zi", tag="zres", bufs=3)
            nc.sync.dma_start(out=zi, in_=z_in[tp * 128:(tp + 1) * 128, :])
            transpose_zrow(zi, tp, zt8, dtype_scale=SZ)

        g0_prefetched = [False]
        for it in range(ITERS):
            # ======== attention ========
            if not g0_prefetched[0]:
                for _, u in qkv_units(0):
                    u()
            g0_prefetched[0] = False
            for g in range(4):
                if g < 3:
                    emit_wgt_dma(g + 1)
                    fillers = deque(qkv_units(g + 1))
                else:
                    fillers = deque()
                for hp in range(2):
                    for hh in range(2):
                        attention_pass(g, hp, hh, fillers)
                while fillers:
                    fillers.popleft()[1]()
            while pending_tail:
                pending_tail.pop()()
            while pending_norm:
                pending_norm.pop()()

            # ======== out-proj + residual + LN1 ========
            if it == 0:
                for dp in range(ND):
                    nc.sync.dma_start(out=wo_sb[:, dp, :],
                                      in_=wo[dp * 128:(dp + 1) * 128, :])
            z_prev = z_in if it == 0 else z_ln2[it - 1]
            dst = z_out if it == ITERS - 1 else z_ln1[it]
            for tp in range(NT1):
                acc = psum.tile([128, 2, 512], F32, name="aao", tag="sc")
                for hep in range(ND):
                    for dq in range(2):
                        nc.tensor.matmul(acc[:, dq, :],
                                         lhsT=outcat[:, hep, tp * 128:(tp + 1) * 128],
                                         rhs=wo_sb[:, hep, dq * 512:(dq + 1) * 512],
                                         start=(hep == 0), stop=(hep == ND - 1))
                zp = work.tile([128, D], F32, name="zp", tag="zres", bufs=3)
                nc.sync.dma_start(out=zp, in_=z_prev[tp * 128:(tp + 1) * 128, :])
                ln_in = work.tile([128, D], F32, name="ln_in", tag="ln_in", bufs=3)
                for dq in range(2):
                    nc.vector.tensor_add(out=ln_in[:, dq * 512:(dq + 1) * 512],
                                         in0=zp[:, dq * 512:(dq + 1) * 512],
                                         in1=acc[:, dq, :])
                z_new = work.tile([128, D], F32, name="z_new", tag="z_new", bufs=3)
                layernorm_tile(ln_in, z_new)
                nc.sync.dma_start(out=dst[tp * 128:(tp + 1) * 128, :], in_=z_new)
                if it < ITERS - 1:
                    transpose_zrow(z_new, tp, z1t)

            if it == ITERS - 1:
                break

            # ======== FFN ========
            with tc.tile_pool(name="htp", bufs=1) as ht_p, \
                 tc.tile_pool(name="w1p", bufs=2) as w1_p, \
                 tc.tile_pool(name="w2p", bufs=3) as w2_p:
                emit_wgt_dma(0)  # prefetch next iteration's first group
                for th in range(NT5):
                    ts0 = th * 512
                    ht = ht_p.tile([128, NF, 512], BF16, name="ht", tag="ht")
                    for fblk in range(8):
                        w1c = w1_p.tile([128, ND, 512], BF16, name="w1c", tag="w1c")
                        for dp in range(ND):
                            nc.sync.dma_start(out=w1c[:, dp, :],
                                              in_=w1[dp * 128:(dp + 1) * 128,
                                                     fblk * 512:(fblk + 1) * 512])
                        for fi in range(4):
                            fc = fblk * 4 + fi
                            acc = psum.tile([128, 512], F32, name="ah", tag="acc")
                            for dp in range(ND):
                                nc.tensor.matmul(acc,
                                                 lhsT=w1c[:, dp, fi * 128:(fi + 1) * 128],
                                                 rhs=z1t[:, dp, ts0:ts0 + 512],
                                                 start=(dp == 0), stop=(dp == ND - 1))
                            nc.scalar.activation(out=ht[:, fc, :], in_=acc, func=AF.Relu)
                    # h2: stream w2 once per th; 4 t-chunk accumulators spread
                    # across the psum tags (sc, sc, pv, acc+acc) = 8 banks.
                    acc_sc0 = psum.tile([128, 2, 512], F32, name="af0", tag="sc")
                    acc_sc1 = psum.tile([128, 2, 512], F32, name="af1", tag="sc")
                    acc_pv = psum.tile([128, 2, 512], F32, name="af2", tag="pv", bufs=1)
                    acc_a0 = psum.tile([128, 512], F32, name="af3a", tag="acc")
                    acc_a1 = psum.tile([128, 512], F32, name="af3b", tag="acc")
                    acc_of = {0: (lambda dq: acc_sc0[:, dq, :]),
                              1: (lambda dq: acc_sc1[:, dq, :]),
                              2: (lambda dq: acc_pv[:, dq, :]),
                              3: (lambda dq: (acc_a0, acc_a1)[dq][:, :])}
                    for fc in range(NF):
                        w2c = w2_p.tile([128, D], BF16, name="w2c", tag="w2c")
                        nc.sync.dma_start(out=w2c, in_=w2[fc * 128:(fc + 1) * 128, :])
                        for ti in range(4):
                            for dq in range(2):
                                nc.tensor.matmul(acc_of[ti](dq),
                                                 lhsT=ht[:, fc, ti * 128:(ti + 1) * 128],
                                                 rhs=w2c[:, dq * 512:(dq + 1) * 512],
                                                 start=(fc == 0), stop=(fc == NF - 1))
                    for ti in range(4):
                        tp = th * 4 + ti
                        zp = work.tile([128, D], F32, name="zp2", tag="zres", bufs=3)
                        nc.sync.dma_start(out=zp, in_=z_ln1[it][tp * 128:(tp + 1) * 128, :])
                        ln_in = work.tile([128, D], F32, name="ln_in2", tag="ln_in", bufs=3)
                        for dq in range(2):
                            nc.vector.tensor_add(out=ln_in[:, dq * 512:(dq + 1) * 512],
                                                 in0=zp[:, dq * 512:(dq + 1) * 512],
                                                 in1=acc_of[ti](dq))
                        z_new = work.tile([128, D], F32, name="z_new2", tag="z_new", bufs=3)
                        layernorm_tile(ln_in, z_new)
                        nc.sync.dma_start(out=z_ln2[it][tp * 128:(tp + 1) * 128, :], in_=z_new)
                        transpose_zrow(z_new, tp, zt8, dtype_scale=SZ)

    nc.compile()
    return nc


def _prep_w_hilo(w):
    """[D, D] f32 -> [D, 2, D] fp8 hi/lo at scale SW."""
    ws = np.asarray(w, dtype=np.float32) * SW
    hi = ws.astype(NPF8)
    lo = (ws - hi.astype(np.float32)).astype(NPF8)
    return np.ascontiguousarray(np.stack([hi, lo], axis=1))


def kernel(**inputs):
    z = np.asarray(inputs["z"], dtype=np.float32)
    for nm in ("bq", "bk", "bv", "bo", "b1", "b2", "be1", "be2"):
        assert not np.any(np.asarray(inputs[nm])), f"{nm} must be zero (specialized kernel)"
    for nm in ("g1", "g2"):
        assert np.all(np.asarray(inputs[nm]) == 1.0), f"{nm} must be ones (specialized kernel)"

    def flat(w):
        return np.asarray(w).transpose(1, 0, 2).reshape(D, D).astype(np.float32)

    wq_ = _prep_w_hilo(flat(inputs["Wq"]))
    wk_ = _prep_w_hilo(flat(inputs["Wk"]))
    wv_ = _prep_w_hilo(flat(inputs["Wv"]))
    wo_ = np.ascontiguousarray(np.asarray(inputs["Wo"], dtype=np.float32).astype(ml_dtypes.bfloat16))
    w1_ = np.ascontiguousarray(np.asarray(inputs["W1"], dtype=np.float32).astype(ml_dtypes.bfloat16))
    w2_ = np.ascontiguousarray(np.asarray(inputs["W2"], dtype=np.float32).astype(ml_dtypes.bfloat16))

    T = z.shape[1]
    if T not in _CACHE:
        _CACHE[T] = build(T)
    nc = _CACHE[T]

    in_maps = [{"z_in": np.ascontiguousarray(z[c]), "wq8": wq_, "wk8": wk_, "wv8": wv_,
                "wo": wo_, "w1": w1_, "w2": w2_} for c in range(B)]
    res = run_bass_kernel_spmd(nc, in_maps, core_ids=list(range(B)))
    return np.stack([res.results[c]["z_out"] for c in range(B)]).astype(np.float32)
